# revision 18
# baseline (speedup 1.0000x reference)
"""CoAttention forward on 8 TRN2 NeuronCores — layout-B restructure.

Data-parallel over batch B=64 (8 batches/core). All heavy products run as
f16 3-pass (hh + h*lo + lo*h ~ 22-bit values) with hi/lo f16 storage of
intermediates; G_v/G_q assemble their direct term exactly in f32 PSUM, so
only the cross terms (t2', S) pay 2-pass hi/lo cost. Logits stay f32.

Per batch b (Q [512,1024], V [196,1024], D=1024):
  U    = W_b V^T                [D(e), NV]  3-pass, stored hi/lo f16
  C    = tanh(Q U)              [NQ, NV]    3-pass, stored f16 (tanh-saturated)
  CT   = C^T                    PE f16 transposes
  per d-half (512):
    WvVT = V W_v^T              [NV, d]     3-pass -> psum kept + hi/lo sbuf
    per q-chunk: G_q^T = C WvVT(2-pass hi/lo) ++ Q W_q^T(3-pass, psum)
                 -> wqqt hi/lo sbuf, H_q = tanh f16
    G_v^T = WvVT(psum) ++ C^T wqqt (2-pass)  -> H_v = tanh f16
  h_v/h_q: DVE dot (H f16 * w-broadcast f16, accum f32) -> PE f32 col
  transpose -> softmax f32 -> a broadcast via PE -> v_hat/q_hat DVE STT.
"""
import numpy as np

import concourse.bass as bass
import concourse.mybir as mybir
import concourse.tile as tile
from concourse import bacc
from concourse.bass_utils import run_bass_kernel_spmd
from concourse.masks import make_identity

AF = mybir.ActivationFunctionType
ALU = mybir.AluOpType
AX = mybir.AxisListType
F32 = mybir.dt.float32
F16 = mybir.dt.float16

B, NV, NQ, D = 64, 196, 512, 1024
NCORES = 8
NB = B // NCORES
KD = D // 128             # 8 feature k-chunks
MQ = NQ // 128            # 4 q-chunks
NV1 = NV - 128            # 68 rows in second v-chunk
VROWS = (128, NV1)


def build(nb=NB):
    nc = bacc.Bacc(None, target_bir_lowering=False)

    QTh_d = nc.dram_tensor("QTh", [nb, D, NQ], F16, kind="ExternalInput")
    QTl_d = nc.dram_tensor("QTl", [nb, D, NQ], F16, kind="ExternalInput")
    VTh_d = nc.dram_tensor("VTh", [nb, D, NV], F16, kind="ExternalInput")
    VTl_d = nc.dram_tensor("VTl", [nb, D, NV], F16, kind="ExternalInput")
    WbTh_d = nc.dram_tensor("WbTh", [D, D], F16, kind="ExternalInput")
    WbTl_d = nc.dram_tensor("WbTl", [D, D], F16, kind="ExternalInput")
    WqTh_d = nc.dram_tensor("WqTh", [D, D], F16, kind="ExternalInput")
    WqTl_d = nc.dram_tensor("WqTl", [D, D], F16, kind="ExternalInput")
    WvTh_d = nc.dram_tensor("WvTh", [D, D], F16, kind="ExternalInput")
    WvTl_d = nc.dram_tensor("WvTl", [D, D], F16, kind="ExternalInput")
    whv_d = nc.dram_tensor("whv", [1, D], F16, kind="ExternalInput")
    whq_d = nc.dram_tensor("whq", [1, D], F16, kind="ExternalInput")
    OV_d = nc.dram_tensor("OV", [nb, D], F32, kind="ExternalOutput")
    OQ_d = nc.dram_tensor("OQ", [nb, D], F32, kind="ExternalOutput")

    with tile.TileContext(nc) as tc:
        with (
            tc.tile_pool(name="wsb", bufs=1) as wsb,
            tc.tile_pool(name="iop", bufs=2) as iop,
            tc.tile_pool(name="mid", bufs=1) as mid,
            tc.tile_pool(name="sm", bufs=1) as sm,
            tc.tile_pool(name="psp", bufs=1, space="PSUM") as psp,
        ):
            # ---- persistent weights ----
            def wtile(name, src):
                t = wsb.tile([128, KD, D], F16, name=name)
                nc.sync.dma_start(out=t, in_=src.rearrange("(k p) d -> p k d", p=128))
                return t

            def load_inputs(b):
                vth = iop.tile([128, KD, NV], F16, tag="vth", name=f"vth{b}")
                nc.sync.dma_start(out=vth,
                                  in_=VTh_d[b].rearrange("(k p) n -> p k n", p=128))
                vtl = iop.tile([128, KD, NV], F16, tag="vtl", name=f"vtl{b}")
                nc.sync.dma_start(out=vtl,
                                  in_=VTl_d[b].rearrange("(k p) n -> p k n", p=128))
                qth = iop.tile([128, KD, NQ], F16, tag="qth", name=f"qth{b}")
                nc.sync.dma_start(out=qth,
                                  in_=QTh_d[b].rearrange("(k p) n -> p k n", p=128))
                qtl = iop.tile([128, KD, NQ], F16, tag="qtl", name=f"qtl{b}")
                nc.sync.dma_start(out=qtl,
                                  in_=QTl_d[b].rearrange("(k p) n -> p k n", p=128))
                return qth, qtl, vth, vtl

            # wbt + batch-0 inputs first so the U phase can start ASAP
            wbth = wtile("wbth", WbTh_d)
            wbtl = wtile("wbtl", WbTl_d)
            inp0 = load_inputs(0)
            wqth = wtile("wqth", WqTh_d)
            wqtl = wtile("wqtl", WqTl_d)
            wvth = wtile("wvth", WvTh_d)
            wvtl = wtile("wvtl", WvTl_d)
            identh = wsb.tile([128, 128], F16)
            make_identity(nc, identh)
            identf = wsb.tile([128, 128], F32)
            make_identity(nc, identf)
            ones16 = wsb.tile([1, 128], F16)
            nc.vector.memset(ones16, 1.0)

            # broadcast w_hv / w_hq rows to [128, D] f16
            whv_r16 = wsb.tile([1, D], F16)
            nc.sync.dma_start(out=whv_r16, in_=whv_d[:, :])
            whq_r16 = wsb.tile([1, D], F16)
            nc.sync.dma_start(out=whq_r16, in_=whq_d[:, :])
            whv_b = wsb.tile([128, D], F16)
            whq_b = wsb.tile([128, D], F16)
            for h in range(2):
                hs = slice(h * 512, (h + 1) * 512)
                for bt, row in ((whv_b, whv_r16), (whq_b, whq_r16)):
                    pb = psp.tile([128, 512], F32, tag="puc", bufs=3,
                                  name=f"pbw{h}_{0 if bt is whv_b else 1}")
                    nc.tensor.matmul(pb, ones16, row[:, hs], start=True, stop=True)
                    nc.scalar.copy(bt[:, hs], pb)

            for b in range(nb):
                qth, qtl, vth, vtl = inp0 if b == 0 else load_inputs(b)

                # ---- U = W_b V^T  [e, v], 3-pass, hi/lo ----
                u_h = mid.tile([128, KD, NV], F16, tag="u_h")
                u_l = mid.tile([128, KD, NV], F16, tag="u_l")
                for e in range(KD):
                    es = slice(e * 128, (e + 1) * 128)
                    pu = psp.tile([128, 512], F32, tag="puc", bufs=3, name=f"pu{b}_{e}")
                    n = 0
                    for k in range(KD):
                        for lh, rh in ((wbth, vth), (wbth, vtl), (wbtl, vth)):
                            n += 1
                            nc.tensor.matmul(pu[:, :NV], lh[:, k, es], rh[:, k, :],
                                             start=(n == 1), stop=(n == 3 * KD))
                    nc.scalar.copy(u_h[:, e, :], pu[:, :NV])
                    nc.vector.tensor_sub(u_l[:, e, :], pu[:, :NV], u_h[:, e, :])

                # ---- C = tanh(Q U)  [q, v], 3-pass, f16 ----
                c16 = mid.tile([128, MQ, NV], F16, tag="c16")
                for m in range(MQ):
                    ms = slice(m * 128, (m + 1) * 128)
                    pc = psp.tile([128, 512], F32, tag="puc", bufs=3, name=f"pc{b}_{m}")
                    n = 0
                    for e in range(KD):
                        for lh, rh in ((qth, u_h), (qth, u_l), (qtl, u_h)):
                            n += 1
                            nc.tensor.matmul(pc[:, :NV], lh[:, e, ms], rh[:, e, :],
                                             start=(n == 1), stop=(n == 3 * KD))
                    nc.scalar.activation(c16[:, m, :], pc[:, :NV], AF.Tanh)

                # ---- CT = C^T  [v, q] f16 via 128x128 PE transposes ----
                ct16 = mid.tile([128, 2, NQ], F16, tag="ct16")
                for mv in range(2):
                    rows = VROWS[mv]
                    vs = slice(mv * 128, mv * 128 + rows)
                    for mq in range(MQ):
                        pt = psp.tile([128, 128], F16, tag="pcts", bufs=1,
                                      name=f"pt{b}_{mv}_{mq}")
                        nc.tensor.transpose(pt[:rows, :], c16[:, mq, vs], identh)
                        nc.scalar.copy(ct16[:rows, mv, mq * 128:(mq + 1) * 128],
                                       pt[:rows, :])

                # ---- per d-half: WvVT, G_q^T, G_v^T ----
                wvvt_h = mid.tile([128, 2, D], F16, tag="wvvt_h")
                wvvt_l = mid.tile([128, 2, D], F16, tag="wvvt_l")
                wqqt_h = mid.tile([128, MQ, D], F16, tag="wqqt_h")
                wqqt_l = mid.tile([128, MQ, D], F16, tag="wqqt_l")
                hv16 = mid.tile([128, 2, D], F16, tag="hv16")
                hq16 = mid.tile([128, MQ, D], F16, tag="hq16")
                hvc = mid.tile([128, 2], F32, tag="hvc")
                hqc = mid.tile([128, MQ], F32, tag="hqc")
                scr = mid.tile([128, D], F16, tag="scr")
                for h in range(2):
                    hs = slice(h * 512, (h + 1) * 512)
                    # (a) WvVT chunks into pv psum (kept open for (c))
                    pv_t = []
                    for mv in range(2):
                        rows = VROWS[mv]
                        vs = slice(mv * 128, mv * 128 + rows)
                        pvt = psp.tile([128, 512], F32, tag="pv", bufs=2,
                                       name=f"pv{b}_{h}_{mv}")
                        n = 0
                        for k in range(KD):
                            for lh, rh in ((vth, wvth), (vth, wvtl), (vtl, wvth)):
                                n += 1
                                nc.tensor.matmul(pvt[:rows, :], lh[:, k, vs],
                                                 rh[:, k, hs],
                                                 start=(n == 1), stop=False)
                        nc.scalar.copy(wvvt_h[:rows, mv, hs], pvt[:rows, :])
                        nc.vector.tensor_sub(wvvt_l[:rows, mv, hs], pvt[:rows, :],
                                             wvvt_h[:rows, mv, hs])
                        pv_t.append(pvt)
                    # (b) per q-chunk: G_q^T = S(2-pass) + WqQT(3-pass)
                    for mq in range(MQ):
                        ms = slice(mq * 128, (mq + 1) * 128)
                        pqt = psp.tile([128, 512], F32, tag="pq", bufs=2,
                                       name=f"pq{b}_{h}_{mq}")
                        nm = 0
                        for k in range(KD):
                            for lh, rh in ((qth, wqth), (qth, wqtl), (qtl, wqth)):
                                nm += 1
                                nc.tensor.matmul(pqt, lh[:, k, ms], rh[:, k, hs],
                                                 start=(nm == 1), stop=False)
                        # snapshot WqQT (hi/lo) before S accumulates on top
                        nc.scalar.copy(wqqt_h[:, mq, hs], pqt)
                        nc.vector.tensor_sub(wqqt_l[:, mq, hs], pqt,
                                             wqqt_h[:, mq, hs])
                        n = 0
                        for mv in range(2):
                            rows = VROWS[mv]
                            for rh in (wvvt_h, wvvt_l):
                                n += 1
                                nc.tensor.matmul(pqt, ct16[:rows, mv, ms],
                                                 rh[:rows, mv, hs],
                                                 start=False, stop=(n == 4))
                        nc.scalar.activation(hq16[:, mq, hs], pqt, AF.Tanh)
                        if h == 1:
                            nc.vector.scalar_tensor_tensor(
                                out=scr, in0=hq16[:, mq, :], scalar=1.0,
                                in1=whq_b, op0=ALU.mult, op1=ALU.mult,
                                accum_out=hqc[:, mq:mq + 1])
                    # (c) G_v^T: t2' accumulates onto WvVT psum
                    for mv in range(2):
                        rows = VROWS[mv]
                        vs = slice(mv * 128, mv * 128 + rows)
                        n = 0
                        for mq in range(MQ):
                            for rh in (wqqt_h, wqqt_l):
                                n += 1
                                nc.tensor.matmul(pv_t[mv][:rows, :],
                                                 c16[:, mq, vs], rh[:, mq, hs],
                                                 start=False, stop=(n == 2 * MQ))
                        nc.scalar.activation(hv16[:rows, mv, hs],
                                             pv_t[mv][:rows, :], AF.Tanh)
                        if h == 1:
                            nc.vector.scalar_tensor_tensor(
                                out=scr[:rows, :], in0=hv16[:rows, mv, :],
                                scalar=1.0, in1=whv_b[:rows, :],
                                op0=ALU.mult, op1=ALU.mult,
                                accum_out=hvc[:rows, mv:mv + 1])

                # ---- logits: PE f32 col transpose ----
                hps_v = psp.tile([128, 512], F32, tag="puc", bufs=3, name=f"hpv{b}")
                for mv in range(2):
                    rows = VROWS[mv]
                    nc.tensor.transpose(hps_v[0:1, mv * 128:mv * 128 + rows],
                                        hvc[:rows, mv:mv + 1], identf[:rows, :rows])
                hps_q = psp.tile([128, 512], F32, tag="puc", bufs=3, name=f"hpq{b}")
                for mq in range(MQ):
                    nc.tensor.transpose(hps_q[0:1, mq * 128:(mq + 1) * 128],
                                        hqc[:, mq:mq + 1], identf)

                # ---- softmax + broadcast ----
                def softmax_bcast(h_ps, n, tagp):
                    negm = sm.tile([1, 1], F32, tag=f"negm{tagp}")
                    nc.vector.reduce_max(negm, h_ps[0:1, :n], axis=AX.X, negate=True)
                    ex = sm.tile([1, n], F32, tag=f"ex{tagp}")
                    ssum = sm.tile([1, 1], F32, tag=f"ssum{tagp}")
                    nc.scalar.activation(ex, h_ps[0:1, :n], AF.Exp, bias=negm,
                                         accum_out=ssum)
                    rs = sm.tile([1, 1], F32, tag=f"rs{tagp}")
                    nc.vector.reciprocal(rs, ssum)
                    ex16 = sm.tile([1, n], F16, tag=f"ex16{tagp}")
                    nc.scalar.mul(ex16, ex, rs)
                    ab_ps = psp.tile([128, 512], F32, tag="puc", bufs=3,
                                     name=f"abps{tagp}{b}")
                    nc.tensor.matmul(ab_ps[:, :n], ones16, ex16, start=True, stop=True)
                    ab = sm.tile([128, n], F32, tag=f"ab{tagp}")
                    nc.scalar.copy(ab, ab_ps[:, :n])
                    return ab

                av_b = softmax_bcast(hps_v, NV, "v")
                aq_b = softmax_bcast(hps_q, NQ, "q")

                # ---- v_hat / q_hat ----
                vhat_sb = sm.tile([128, KD], F32, tag="vhat")
                qhat_sb = sm.tile([128, KD], F32, tag="qhat")
                for k in range(KD):
                    nc.vector.scalar_tensor_tensor(
                        out=scr[:, :NV], in0=vth[:, k, :], scalar=1.0,
                        in1=av_b, op0=ALU.mult, op1=ALU.mult,
                        accum_out=vhat_sb[:, k:k + 1])
                for k in range(KD):
                    nc.vector.scalar_tensor_tensor(
                        out=scr[:, :NQ], in0=qth[:, k, :], scalar=1.0,
                        in1=aq_b, op0=ALU.mult, op1=ALU.mult,
                        accum_out=qhat_sb[:, k:k + 1])
                nc.sync.dma_start(out=OV_d[b].rearrange("(k p) -> p k", p=128), in_=vhat_sb)
                nc.sync.dma_start(out=OQ_d[b].rearrange("(k p) -> p k", p=128), in_=qhat_sb)

    nc.finalize()
    return nc


_BUILT = {}


def _split(x):
    hi = x.astype(np.float16)
    lo = (x - hi.astype(np.float32)).astype(np.float16)
    return np.ascontiguousarray(hi), np.ascontiguousarray(lo)


def kernel(V, Q, W_b, W_v, W_q, w_hv, w_hq, _trace=False):
    V = np.asarray(V, dtype=np.float32)
    Q = np.asarray(Q, dtype=np.float32)
    nb = B // NCORES
    QTh, QTl = _split(Q.transpose(0, 2, 1))      # [B, D, NQ] f16
    VTh, VTl = _split(V.transpose(0, 2, 1))      # [B, D, NV] f16
    WbTh, WbTl = _split(np.asarray(W_b, dtype=np.float32).T)
    WqTh, WqTl = _split(np.asarray(W_q, dtype=np.float32).T)
    WvTh, WvTl = _split(np.asarray(W_v, dtype=np.float32).T)
    whv = np.ascontiguousarray(np.asarray(w_hv, dtype=np.float32).reshape(1, D).astype(np.float16))
    whq = np.ascontiguousarray(np.asarray(w_hq, dtype=np.float32).reshape(1, D).astype(np.float16))

    if nb not in _BUILT:
        _BUILT[nb] = build(nb)
    nc = _BUILT[nb]

    in_maps = []
    for c in range(NCORES):
        sl = slice(c * nb, (c + 1) * nb)
        in_maps.append({
            "QTh": np.ascontiguousarray(QTh[sl]), "QTl": np.ascontiguousarray(QTl[sl]),
            "VTh": np.ascontiguousarray(VTh[sl]), "VTl": np.ascontiguousarray(VTl[sl]),
            "WbTh": WbTh, "WbTl": WbTl, "WqTh": WqTh, "WqTl": WqTl,
            "WvTh": WvTh, "WvTl": WvTl, "whv": whv, "whq": whq,
        })

    out = run_bass_kernel_spmd(nc, in_maps, core_ids=list(range(NCORES)),
                               trace=_trace)
    v_hat = np.concatenate([out.results[c]["OV"] for c in range(NCORES)], axis=0)
    q_hat = np.concatenate([out.results[c]["OQ"] for c in range(NCORES)], axis=0)
    if _trace:
        kernel._last_exec_ns = out.exec_time_ns
        kernel._last_results = out
    return (v_hat, q_hat)
